# revision 11
# baseline (speedup 1.0000x reference)
"""Cross-modal center contrastive loss on 8 Trainium2 NeuronCores.

Math: every entry of the reference's 4096x4096 distance matrix depends only on
the *class pair* (targets[i], targets[j]), because centersR[i] = class_mean[t_i]
and centers[i] = centers_param[t_i].  The loss therefore collapses to a C x C
computation weighted by class counts:

    loss = (1/N^2) * sum_m [  sum_a cnt_a^2 * sq_m[a, a]
                            + sum_{a != b} cnt_a * cnt_b * relu(0.5 - d_m[a, b])^2 ]

with sq_m[a, b] = clip(||mean_m[a] - centers_param[b]||^2, 1e-12), d = sqrt(sq).

Device plan (SPMD over 8 cores):
  phase 1: batch-shard rows 8-way; per-core segment sums via one-hot matmul
           (f32r, exact for 0/1 weights); partial counts via ones-matmul.
  comm:    ReduceScatter the stacked per-modality sums [800, 512] so core k
           owns a 100-class chunk of one modality; ReduceScatter counts
           (stacked twice) for own-chunk counts; AllReduce counts for the
           global count row.
  phase 2: per-core 100 x 395 distance block (means via inverse-count scaling,
           -2*dot via PE with transposed means, cnorm folded in via a K=1
           matmul), margin/contrastive weighting, count-weighted reduction to
           one scalar partial per core.
  gather:  host sums the 8 partials.
"""

import numpy as np

from concourse import bacc, bass, mybir
from concourse import tile
from concourse.bass_utils import run_bass_kernel_spmd

F32 = mybir.dt.float32
F32R = mybir.dt.float32r
AT = mybir.ActivationFunctionType
OP = mybir.AluOpType
AX = mybir.AxisListType

NCORES = 8
N = 4096          # batch rows
D = 512           # feature dim
CR = 395          # real number of classes
C1 = 400          # padded classes per modality (divisible by 8 and 4)
MCH = C1 // 4     # class chunk owned per core after reduce-scatter
RPC = N // NCORES # rows per core
P = 128           # SBUF partitions
RT = RPC // P     # row tiles per core
INV_N2 = 1.0 / (N * N)  # 2^-24, exact in f32
GROUPS = [list(range(NCORES))]


def _r(ap):
    return ap.bitcast(F32R)


def build_nc() -> bass.Bass:  # noqa: returns Bacc
    nc = bacc.Bacc(None, num_devices=NCORES)

    f1 = nc.dram_tensor("f1", [RPC, D], F32, kind="ExternalInput")
    f2 = nc.dram_tensor("f2", [RPC, D], F32, kind="ExternalInput")
    tgt = nc.dram_tensor("tgt", [RPC, 1], F32, kind="ExternalInput")
    rowcls = nc.dram_tensor("rowcls", [MCH, 1], F32, kind="ExternalInput")
    cT = nc.dram_tensor("cT", [D, C1], F32, kind="ExternalInput")
    out_part = nc.dram_tensor("out_part", [1, 1], F32, kind="ExternalOutput")

    with tile.TileContext(nc, num_cores=NCORES) as tc:
        with (
            tc.tile_pool(name="const", bufs=1) as cb,
            tc.tile_pool(name="sb", bufs=1) as sb,
            tc.tile_pool(name="stage", bufs=3) as stg,
            tc.tile_pool(name="ps", bufs=1, space="PSUM") as ps,
            tc.tile_pool(name="dram", bufs=1, space="DRAM") as dr,
        ):
            # ---- internal DRAM (collective bounce buffers) ----
            in_s = dr.tile([2 * C1, D], F32)
            in_cnt = dr.tile([2 * C1, 1], F32)
            s_own = dr.tile([MCH, D], F32)
            cnt_own_d = dr.tile([MCH, 1], F32)
            cnt_all_d = dr.tile([C1, 1], F32, addr_space="Shared")

            # ---- constants ----
            iota = cb.tile([P, C1], F32)  # iota[p, j] = j
            nc.gpsimd.iota(iota[:], pattern=[[1, C1]], base=0,
                           channel_multiplier=0,
                           allow_small_or_imprecise_dtypes=True)
            ones_col = cb.tile([P, 1], F32R)
            nc.vector.tensor_scalar(ones_col[:], iota[:, 0:1], -1.0, None,
                                    OP.is_gt)
            ones_row = cb.tile([1, P], F32R)
            nc.vector.tensor_scalar(ones_row[:], iota[0:1, 0:P], -1.0, None,
                                    OP.is_gt)
            half_col = cb.tile([P, 1], F32)
            nc.vector.memset(half_col[:], 0.5)
            ident = cb.tile([P, P], F32)
            nc.gpsimd.memset(ident[:], 0.0)
            nc.gpsimd.affine_select(out=ident[:], in_=ident[:],
                                    compare_op=OP.not_equal, fill=1.0,
                                    base=0, pattern=[[-1, P]],
                                    channel_multiplier=1)

            # ---- phase 1: load shards, one-hot, partial segment sums ----
            tgt_sb = sb.tile([P, RT], F32)
            nc.gpsimd.dma_start(
                tgt_sb[:], tgt.rearrange("(r p) one -> p (r one)", r=RT))
            f_tiles = {1: [], 2: []}
            oh_tiles = []
            for r in range(RT):
                ft1 = sb.tile([P, D], F32R, name=f"ft1_{r}")
                nc.sync.dma_start(ft1[:], f1[r * P:(r + 1) * P, :].bitcast(F32R))
                f_tiles[1].append(ft1)
                ft2 = sb.tile([P, D], F32R, name=f"ft2_{r}")
                nc.sync.dma_start(ft2[:], f2[r * P:(r + 1) * P, :].bitcast(F32R))
                f_tiles[2].append(ft2)
                oh = sb.tile([P, C1], F32R, name=f"oh_{r}")
                nc.vector.tensor_scalar(oh[:], iota[:], tgt_sb[:, r:r + 1],
                                        None, OP.is_equal)
                oh_tiles.append(oh)

            # counts partial: cnt[c] = sum_rows onehot[:, c]
            cnt_ps = ps.tile([1, C1], F32, name="cnt_ps")
            for r in range(RT):
                nc.tensor.matmul(cnt_ps[:], ones_col[:], oh_tiles[r][:],
                                 start=(r == 0), stop=(r == RT - 1))
            cnt_sb = sb.tile([1, C1], F32)
            nc.vector.tensor_copy(cnt_sb[:], cnt_ps[:])
            nc.gpsimd.dma_start(in_cnt[0:C1, :], cnt_sb[:])
            nc.gpsimd.dma_start(in_cnt[C1:2 * C1, :], cnt_sb[:])

            # partial sums: sums_m[c, :] = sum_{rows with t=c} feat_m[row, :]
            for mod in (1, 2):
                for m in range(4):
                    sp = ps.tile([MCH, D], F32, name="sp", tag="sp", bufs=2)
                    for r in range(RT):
                        nc.tensor.matmul(
                            sp[:],
                            oh_tiles[r][:, m * MCH:(m + 1) * MCH],
                            f_tiles[mod][r][:],
                            start=(r == 0), stop=(r == RT - 1))
                    ssb = stg.tile([MCH, D], F32, name="ssb", tag="ssb")
                    nc.vector.tensor_copy(ssb[:], sp[:])
                    base = (mod - 1) * C1 + m * MCH
                    nc.sync.dma_start(in_s[base:base + MCH, :], ssb[:])

            # ---- collectives ----
            nc.gpsimd.collective_compute(
                "ReduceScatter", OP.add, replica_groups=GROUPS,
                ins=[in_cnt.opt()], outs=[cnt_own_d.opt()])
            nc.gpsimd.collective_compute(
                "AllReduce", OP.add, replica_groups=GROUPS,
                ins=[in_cnt[0:C1, :].opt()], outs=[cnt_all_d.opt()])
            nc.gpsimd.collective_compute(
                "ReduceScatter", OP.add, replica_groups=GROUPS,
                ins=[in_s.opt()], outs=[s_own.opt()])

            # ---- phase 2: own 100-class chunk vs all 395 centers ----
            so = sb.tile([MCH, D], F32)
            nc.sync.dma_start(so[:], s_own[:])
            co = sb.tile([MCH, 1], F32)
            nc.gpsimd.dma_start(co[:], cnt_own_d[:])
            ca = sb.tile([1, C1], F32)
            nc.gpsimd.dma_start(ca[:], cnt_all_d[:])
            rc = sb.tile([MCH, 1], F32)
            nc.gpsimd.dma_start(rc[:], rowcls[:])
            ct_tiles = []
            for j in range(4):
                ctj = sb.tile([P, C1], F32R, name=f"ct_{j}")
                nc.sync.dma_start(ctj[:], cT[j * P:(j + 1) * P, :].bitcast(F32R))
                ct_tiles.append(ctj)

            clamped = sb.tile([MCH, 1], F32)
            nc.vector.tensor_scalar(clamped[:], co[:], 1.0, None, OP.max)
            inv = sb.tile([MCH, 1], F32)
            nc.vector.reciprocal(inv[:], clamped[:])
            inv_n2 = sb.tile([MCH, 1], F32)
            nc.vector.tensor_scalar(inv_n2[:], inv[:], -2.0, None, OP.mult)
            inv2 = sb.tile([MCH, 1], F32)
            nc.vector.tensor_tensor(inv2[:], inv[:], inv[:], OP.mult)
            snorm = sb.tile([MCH, 1], F32)
            sqtmp = sb.tile([MCH, D], F32)
            nc.scalar.activation(sqtmp[:], so[:], AT.Square, accum_out=snorm[:])
            mnorm = sb.tile([MCH, 1], F32)
            nc.vector.tensor_tensor(mnorm[:], snorm[:], inv2[:], OP.mult)
            # means scaled by -2 (so the PE dot matmul directly yields -2*dot)
            means_n2 = sb.tile([MCH, D], F32)
            nc.vector.tensor_scalar(means_n2[:], so[:], inv_n2[:], None, OP.mult)

            mt_tiles = []
            for j in range(4):
                tp = ps.tile([P, MCH], F32, name="tp", tag="tp", bufs=2)
                nc.tensor.transpose(tp[:], means_n2[:, j * P:(j + 1) * P],
                                    ident[0:MCH, 0:MCH])
                mt = sb.tile([P, MCH], F32R, name=f"mt_{j}")
                nc.vector.tensor_copy(mt[:], tp[:])
                mt_tiles.append(mt)

            cn_ps = ps.tile([1, C1], F32, name="cn_ps")
            for j in range(4):
                csq = stg.tile([P, C1], F32R, name="csq", tag="csq")
                nc.vector.tensor_tensor(csq[:], ct_tiles[j][:], ct_tiles[j][:],
                                        OP.mult)
                nc.tensor.matmul(cn_ps[:], ones_col[:], csq[:],
                                 start=(j == 0), stop=(j == 3))
            cnorm = sb.tile([1, C1], F32R)
            nc.vector.tensor_copy(cnorm[:], cn_ps[:])

            sq_ps = ps.tile([MCH, C1], F32, name="sq_ps")
            for j in range(4):
                nc.tensor.matmul(sq_ps[:], mt_tiles[j][:], ct_tiles[j][:],
                                 start=(j == 0), stop=False)
            nc.tensor.matmul(sq_ps[:], ones_row[:, 0:MCH], cnorm[:],
                             start=False, stop=True)

            # sq = max(-2*dot + cnorm + mnorm, 1e-12)
            sq_sb = sb.tile([MCH, C1], F32)
            nc.vector.tensor_scalar(sq_sb[:], sq_ps[:], mnorm[:], 1e-12,
                                    OP.add, OP.max)
            d_sb = sb.tile([MCH, C1], F32)
            nc.scalar.activation(d_sb[:], sq_sb[:], AT.Sqrt)
            neg = sb.tile([MCH, C1], F32)
            nc.scalar.activation(neg[:], d_sb[:], AT.Relu,
                                 bias=half_col[0:MCH, :], scale=-1.0)
            negsq = sb.tile([MCH, C1], F32)
            nc.vector.tensor_tensor(negsq[:], neg[:], neg[:], OP.mult)

            # T = sq on the global diagonal (j == own class), negsq elsewhere
            mask = sb.tile([MCH, C1], mybir.dt.uint8)
            nc.vector.tensor_scalar(mask[:], iota[0:MCH, :], rc[:], None,
                                    OP.is_equal)
            tsel = sb.tile([MCH, C1], F32)
            nc.vector.select(tsel[:], mask[:], sq_sb[:], negsq[:])

            # S_partial = sum_b cnt_all[b] * sum_a cnt_own[a] * T[a, b]
            cs_ps = ps.tile([1, C1], F32, name="cs_ps")
            nc.tensor.matmul(cs_ps[:], co[:], tsel[:],
                             start=True, stop=True)
            wrow = sb.tile([1, C1], F32)
            nc.vector.tensor_tensor(wrow[:], cs_ps[:], ca[:], OP.mult)
            acc = sb.tile([1, 1], F32)
            nc.vector.tensor_reduce(acc[:], wrow[:], AX.X, OP.add)
            part = sb.tile([1, 1], F32)
            nc.vector.tensor_scalar(part[:], acc[:], INV_N2, None, OP.mult)
            nc.gpsimd.dma_start(out_part[:], part[:])

    if not nc.is_finalized():
        nc.finalize()
    return nc


_NC_CACHE = None


def _get_nc() -> bass.Bass:
    global _NC_CACHE
    if _NC_CACHE is None:
        _NC_CACHE = build_nc()
    return _NC_CACHE


def make_in_maps(modal1_inputs, modal2_inputs, centers_param, targets):
    m1 = np.ascontiguousarray(np.asarray(modal1_inputs, dtype=np.float32))
    m2 = np.ascontiguousarray(np.asarray(modal2_inputs, dtype=np.float32))
    cp = np.asarray(centers_param, dtype=np.float32)
    t = np.asarray(targets).astype(np.float32).reshape(N, 1)
    cT = np.zeros((D, C1), dtype=np.float32)
    cT[:, :CR] = cp.T
    in_maps = []
    for k in range(NCORES):
        rows = slice(k * RPC, (k + 1) * RPC)
        ck = k % 4
        in_maps.append({
            "f1": m1[rows],
            "f2": m2[rows],
            "tgt": np.ascontiguousarray(t[rows]),
            "rowcls": np.arange(ck * MCH, (ck + 1) * MCH,
                                dtype=np.float32).reshape(MCH, 1),
            "cT": cT,
        })
    return in_maps


def run(modal1_inputs, modal2_inputs, centers_param, targets, trace=False):
    nc = _get_nc()
    in_maps = make_in_maps(modal1_inputs, modal2_inputs, centers_param, targets)
    res = run_bass_kernel_spmd(nc, in_maps, list(range(NCORES)), trace=trace)
    parts = np.array([res.results[k]["out_part"][0, 0] for k in range(NCORES)],
                     dtype=np.float32)
    loss = np.array(parts.sum(), dtype=np.float32)
    return loss, res


def kernel(modal1_inputs, modal2_inputs, centers_param, targets):
    loss, _ = run(modal1_inputs, modal2_inputs, centers_param, targets)
    return loss


# revision 13
# speedup vs baseline: 1.7353x; 1.7353x over previous
"""Cross-modal center contrastive loss on 8 Trainium2 NeuronCores.

Math: every entry of the reference's 4096x4096 distance matrix depends only on
the *class pair* (targets[i], targets[j]), because centersR[i] = class_mean[t_i]
and centers[i] = centers_param[t_i].  The loss therefore collapses to a C x C
computation weighted by class counts:

    loss = (1/N^2) * sum_m [  sum_a cnt_a^2 * sq_m[a, a]
                            + sum_{a != b} cnt_a * cnt_b * relu(0.5 - d_m[a, b])^2 ]

with sq_m[a, b] = clip(||mean_m[a] - centers_param[b]||^2, 1e-12), d = sqrt(sq).

Device plan (SPMD over 8 cores, no collectives - on this part the ncfw
first-collective barrier plus three serialized collective ops cost ~70us,
far more than re-reading the features):
  cores 0-3 own modality 1, cores 4-7 own modality 2; core k owns the
  100-class chunk ck = k%4 (classes padded 395->400).
  Each core streams the FULL feature matrix of its modality (8 MB) and
  accumulates, directly in PSUM:
    sums_own [100, 512]  one-hot(own-chunk) matmul against features
    cnt_all  [1, 400]    ones-matmul against full-width one-hot
    cnt_own  [1, 100]    ones-matmul against own-chunk one-hot
  then computes its 100 x 400 distance block (means via inverse-count
  scaling, -2*dot on PE with transposed means, center norms folded in via a
  K=1 matmul), margin/contrastive weighting, and a count-weighted reduction
  to one scalar partial.  Host sums the 8 partials.
"""

import numpy as np

from concourse import bacc, bass, mybir
from concourse import tile
from concourse.bass_utils import run_bass_kernel_spmd

F32 = mybir.dt.float32
F32R = mybir.dt.float32r
U8 = mybir.dt.uint8
AT = mybir.ActivationFunctionType
OP = mybir.AluOpType
AX = mybir.AxisListType

NCORES = 8
N = 4096          # batch rows
D = 512           # feature dim
CR = 395          # real number of classes
C1 = 400          # padded class count (f32r matmul wants free dim % 4 == 0)
MCH = C1 // 4     # class chunk owned per core
P = 128           # SBUF partitions
RT = N // P       # 32 row tiles (full modality per core)
INV_N2 = 1.0 / (N * N)  # 2^-24, exact in f32


def build_nc() -> bass.Bass:
    nc = bacc.Bacc(None, num_devices=NCORES)

    feat = nc.dram_tensor("feat", [N, D], F32, kind="ExternalInput")
    tgt = nc.dram_tensor("tgt", [N, 1], F32, kind="ExternalInput")
    rowcls = nc.dram_tensor("rowcls", [MCH, 1], F32, kind="ExternalInput")
    iown = nc.dram_tensor("iown", [P, MCH], F32, kind="ExternalInput")
    cT = nc.dram_tensor("cT", [D, C1], F32, kind="ExternalInput")
    out_part = nc.dram_tensor("out_part", [1, 1], F32, kind="ExternalOutput")

    with tile.TileContext(nc, num_cores=NCORES) as tc:
        with (
            tc.tile_pool(name="const", bufs=1) as cb,
            tc.tile_pool(name="sb", bufs=1) as sb,
            tc.tile_pool(name="stream", bufs=4) as stm,
            tc.tile_pool(name="ps", bufs=1, space="PSUM") as ps,
        ):
            # ---- constants ----
            iota = cb.tile([P, C1], F32)  # iota[p, j] = j
            nc.gpsimd.iota(iota[:], pattern=[[1, C1]], base=0,
                           channel_multiplier=0,
                           allow_small_or_imprecise_dtypes=True)
            ones_col = cb.tile([P, 1], F32R)
            nc.vector.tensor_scalar(ones_col[:], iota[:, 0:1], -1.0, None,
                                    OP.is_gt)
            ones_row = cb.tile([1, P], F32R)
            nc.vector.tensor_scalar(ones_row[:], iota[0:1, 0:P], -1.0, None,
                                    OP.is_gt)
            half_col = cb.tile([P, 1], F32)
            nc.vector.memset(half_col[:], 0.5)
            ident = cb.tile([P, P], F32)
            nc.gpsimd.memset(ident[:], 0.0)
            nc.gpsimd.affine_select(out=ident[:], in_=ident[:],
                                    compare_op=OP.not_equal, fill=1.0,
                                    base=0, pattern=[[-1, P]],
                                    channel_multiplier=1)

            # warm the ACT LUTs (Square/Sqrt/Relu) while DMA streams
            warm = cb.tile([1, 1], F32)
            nc.scalar.activation(warm[:], half_col[0:1, :], AT.Square)
            nc.scalar.activation(warm[:], warm[:], AT.Sqrt)
            nc.scalar.activation(warm[:], warm[:], AT.Relu,
                                 bias=half_col[0:1, :], scale=-1.0)

            # per-core inputs
            io_sb = sb.tile([P, MCH], F32)   # iota of own classes (by column)
            nc.sync.dma_start(io_sb[:], iown[:])
            rc = sb.tile([MCH, 1], F32)
            nc.gpsimd.dma_start(rc[:], rowcls[:])
            tgt_sb = sb.tile([P, RT], F32)   # targets, column r = rows rP..rP+127
            nc.sync.dma_start(
                tgt_sb[:], tgt.rearrange("(r p) one -> p (r one)", r=RT))
            ct_tiles = []
            for j in range(4):
                ctj = sb.tile([P, C1], F32R, name=f"ct_{j}")
                nc.sync.dma_start(ctj[:], cT[j * P:(j + 1) * P, :].bitcast(F32R))
                ct_tiles.append(ctj)

            # ---- phase 1: stream features, accumulate sums + counts ----
            sums_ps = ps.tile([MCH, D], F32, name="sums_ps")
            cnt_ps = ps.tile([1, C1], F32, name="cnt_ps")
            cno_ps = ps.tile([1, MCH], F32, name="cno_ps")
            for r in range(RT):
                fr = stm.tile([P, D], F32R, name="fr", tag="fr")
                nc.sync.dma_start(fr[:], feat[r * P:(r + 1) * P, :].bitcast(F32R))
                tcol = tgt_sb[:, r:r + 1]
                oh_own = stm.tile([P, MCH], F32R, name="oh_own", tag="oh_own")
                nc.vector.tensor_scalar(oh_own[:], io_sb[:], tcol, None,
                                        OP.is_equal)
                oh_full = stm.tile([P, C1], F32R, name="oh_full", tag="oh_full")
                nc.vector.tensor_scalar(oh_full[:], iota[:], tcol, None,
                                        OP.is_equal)
                nc.tensor.matmul(sums_ps[:], oh_own[:], fr[:],
                                 start=(r == 0), stop=(r == RT - 1))
                nc.tensor.matmul(cnt_ps[:], ones_col[:], oh_full[:],
                                 start=(r == 0), stop=(r == RT - 1))
                nc.tensor.matmul(cno_ps[:], ones_col[:], oh_own[:],
                                 start=(r == 0), stop=(r == RT - 1))

            so = sb.tile([MCH, D], F32)
            nc.vector.tensor_copy(so[:], sums_ps[:])
            ca = sb.tile([1, C1], F32)
            nc.vector.tensor_copy(ca[:], cnt_ps[:])
            cno_row = sb.tile([1, MCH], F32)
            nc.vector.tensor_copy(cno_row[:], cno_ps[:])
            # own-chunk counts as a column: PE transpose of [1, 100]
            cot_ps = ps.tile([MCH, 1], F32, name="cot_ps")
            nc.tensor.transpose(cot_ps[:], cno_row[:], ident[0:1, 0:1])
            co = sb.tile([MCH, 1], F32)
            nc.vector.tensor_copy(co[:], cot_ps[:])

            # ---- phase 2: own 100-class chunk vs all 400 centers ----
            clamped = sb.tile([MCH, 1], F32)
            nc.vector.tensor_scalar(clamped[:], co[:], 1.0, None, OP.max)
            inv = sb.tile([MCH, 1], F32)
            nc.vector.reciprocal(inv[:], clamped[:])
            inv_n2 = sb.tile([MCH, 1], F32)
            nc.vector.tensor_scalar(inv_n2[:], inv[:], -2.0, None, OP.mult)
            inv2 = sb.tile([MCH, 1], F32)
            nc.vector.tensor_tensor(inv2[:], inv[:], inv[:], OP.mult)
            snorm = sb.tile([MCH, 1], F32)
            sqtmp = sb.tile([MCH, D], F32)
            nc.scalar.activation(sqtmp[:], so[:], AT.Square, accum_out=snorm[:])
            mnorm = sb.tile([MCH, 1], F32)
            nc.vector.tensor_tensor(mnorm[:], snorm[:], inv2[:], OP.mult)
            # means scaled by -2 (so the PE dot matmul directly yields -2*dot)
            means_n2 = sb.tile([MCH, D], F32)
            nc.vector.tensor_scalar(means_n2[:], so[:], inv_n2[:], None,
                                    OP.mult)

            mt_tiles = []
            for j in range(4):
                tp = ps.tile([P, MCH], F32, name="tp", tag="tp", bufs=1)
                nc.tensor.transpose(tp[:], means_n2[:, j * P:(j + 1) * P],
                                    ident[0:MCH, 0:MCH])
                mt = sb.tile([P, MCH], F32R, name=f"mt_{j}")
                nc.vector.tensor_copy(mt[:], tp[:])
                mt_tiles.append(mt)

            cn_ps = ps.tile([1, C1], F32, name="cn_ps")
            for j in range(4):
                csq = stm.tile([P, C1], F32R, name="csq", tag="csq")
                nc.vector.tensor_tensor(csq[:], ct_tiles[j][:], ct_tiles[j][:],
                                        OP.mult)
                nc.tensor.matmul(cn_ps[:], ones_col[:], csq[:],
                                 start=(j == 0), stop=(j == 3))
            cnorm = sb.tile([1, C1], F32R)
            nc.vector.tensor_copy(cnorm[:], cn_ps[:])

            sq_ps = ps.tile([MCH, C1], F32, name="sq_ps")
            for j in range(4):
                nc.tensor.matmul(sq_ps[:], mt_tiles[j][:], ct_tiles[j][:],
                                 start=(j == 0), stop=False)
            nc.tensor.matmul(sq_ps[:], ones_row[:, 0:MCH], cnorm[:],
                             start=False, stop=True)

            # sq = max(-2*dot + cnorm + mnorm, 1e-12)
            sq_sb = sb.tile([MCH, C1], F32)
            nc.vector.tensor_scalar(sq_sb[:], sq_ps[:], mnorm[:], 1e-12,
                                    OP.add, OP.max)
            d_sb = sb.tile([MCH, C1], F32)
            nc.scalar.activation(d_sb[:], sq_sb[:], AT.Sqrt)
            neg = sb.tile([MCH, C1], F32)
            nc.scalar.activation(neg[:], d_sb[:], AT.Relu,
                                 bias=half_col[0:MCH, :], scale=-1.0)
            negsq = sb.tile([MCH, C1], F32)
            nc.vector.tensor_tensor(negsq[:], neg[:], neg[:], OP.mult)

            # T = sq on the global diagonal (column == own class), negsq off it
            mask = sb.tile([MCH, C1], U8)
            nc.vector.tensor_scalar(mask[:], iota[0:MCH, :], rc[:], None,
                                    OP.is_equal)
            tsel = sb.tile([MCH, C1], F32)
            nc.vector.select(tsel[:], mask[:], sq_sb[:], negsq[:])

            # S_partial = sum_b cnt_all[b] * sum_a cnt_own[a] * T[a, b]
            cs_ps = ps.tile([1, C1], F32, name="cs_ps")
            nc.tensor.matmul(cs_ps[:], co[:], tsel[:], start=True, stop=True)
            wrow = sb.tile([1, C1], F32)
            nc.vector.tensor_tensor(wrow[:], cs_ps[:], ca[:], OP.mult)
            acc = sb.tile([1, 1], F32)
            nc.vector.tensor_reduce(acc[:], wrow[:], AX.X, OP.add)
            part = sb.tile([1, 1], F32)
            nc.vector.tensor_scalar(part[:], acc[:], INV_N2, None, OP.mult)
            nc.gpsimd.dma_start(out_part[:], part[:])

    if not nc.is_finalized():
        nc.finalize()
    return nc


_NC_CACHE = None


def _get_nc() -> bass.Bass:
    global _NC_CACHE
    if _NC_CACHE is None:
        _NC_CACHE = build_nc()
    return _NC_CACHE


def make_in_maps(modal1_inputs, modal2_inputs, centers_param, targets):
    m1 = np.ascontiguousarray(np.asarray(modal1_inputs, dtype=np.float32))
    m2 = np.ascontiguousarray(np.asarray(modal2_inputs, dtype=np.float32))
    cp = np.asarray(centers_param, dtype=np.float32)
    t = np.asarray(targets).astype(np.float32).reshape(N, 1)
    cT = np.zeros((D, C1), dtype=np.float32)
    cT[:, :CR] = cp.T
    in_maps = []
    for k in range(NCORES):
        ck = k % 4
        cls0 = ck * MCH
        in_maps.append({
            "feat": m1 if k < 4 else m2,
            "tgt": t,
            "rowcls": np.arange(cls0, cls0 + MCH,
                                dtype=np.float32).reshape(MCH, 1),
            "iown": np.broadcast_to(
                np.arange(cls0, cls0 + MCH, dtype=np.float32),
                (P, MCH)).copy(),
            "cT": cT,
        })
    return in_maps


def run(modal1_inputs, modal2_inputs, centers_param, targets, trace=False):
    nc = _get_nc()
    in_maps = make_in_maps(modal1_inputs, modal2_inputs, centers_param, targets)
    res = run_bass_kernel_spmd(nc, in_maps, list(range(NCORES)), trace=trace)
    parts = np.array([res.results[k]["out_part"][0, 0] for k in range(NCORES)],
                     dtype=np.float32)
    loss = np.array(parts.sum(), dtype=np.float32)
    return loss, res


def kernel(modal1_inputs, modal2_inputs, centers_param, targets):
    loss, _ = run(modal1_inputs, modal2_inputs, centers_param, targets)
    return loss
